# revision 1
# baseline (speedup 1.0000x reference)
"""CropAndResize (TF-style, bilinear, extrap=0) on 8 trn2 NeuronCores.

Sharding: data-parallel over batch B=8 (core b owns image[b]); boxes grouped by
their batch index (sharding_hint option 2). Each core:
  Phase A: CHW -> Q layout in DRAM, Q[y, x, r, c] = img[y+r, x, c]
           (paired rows, channels innermost) via PE transposes. 199x200x2x256 f32.
  Phase B: per 128 sample points, one indirect-DMA gather: descriptor s fetches
           the 4KB block Q[ys, xs, :, :] (the 4 bilinear corner pixels x 256ch),
           DVE applies the 4 bilinear weights (per-partition scalars),
           PE transposes [pt, c] -> [c, pt], result accumulates in SBUF,
           two big DMAs write [c, pts] -> out[m, c, 14, 14].
Host only: grouping boxes by box_indices, bilinear index/weight precompute
(O(N*14) floats), and unshard of outputs.
"""
import sys, os, time
sys.path.insert(0, "/opt/trn_rl_repo")
import numpy as np

import concourse.bass as bass
import concourse.bacc as bacc
import concourse.tile as tile
import concourse.mybir as mybir
from concourse.masks import make_identity
import jax
from jax.sharding import Mesh, PartitionSpec
from jax.experimental.shard_map import shard_map
from concourse.bass2jax import _bass_exec_p, install_neuronx_cc_hook, partition_id_tensor

N_CORES = 8
C, H, W = 256, 200, 200
CH, CW = 14, 14
NPT = CH * CW                     # 196 points per box
PX = H * W                        # 40000 pixels
QROWS = PX - W                    # valid block start pixels: ys<=198 -> idx <= 39798

_cache = {}
LAST_EXEC_S = None


def _build(M):
    """Build + compile the SPMD program for M boxes per core. Returns runner."""
    R = (M * NPT + 127) // 128    # gather rounds (128 points each)
    nc = bacc.Bacc("TRN2", target_bir_lowering=False, debug=False, num_devices=N_CORES)
    f32, i32 = mybir.dt.float32, mybir.dt.int32

    img = nc.dram_tensor("img", [C, PX], f32, kind="ExternalInput").ap()
    idxg = nc.dram_tensor("idxg", [128, R], i32, kind="ExternalInput").ap()
    wts = nc.dram_tensor("wts", [128, 4 * R], f32, kind="ExternalInput").ap()
    out = nc.dram_tensor("out", [M, C, NPT], f32, kind="ExternalOutput").ap()
    # Q scratch: flat (H-1)*W*2*C elems; viewed as rows of 512 f32 for the gather
    qflat = nc.dram_tensor("qscratch", [PX * 2 * C], f32, kind="Internal").ap()

    CHUNK = 1024
    nchunks = (PX + CHUNK - 1) // CHUNK

    with tile.TileContext(nc) as tc:
        with tc.tile_pool(name="ident", bufs=1) as ipool:
            ident = ipool.tile([128, 128], f32)
            make_identity(nc, ident[:])

            # ---------------- Phase A: build Q ----------------
            with tc.tile_pool(name="pa_in", bufs=3) as pin, \
                 tc.tile_pool(name="pa_st", bufs=3) as pst, \
                 tc.tile_pool(name="pa_ps", bufs=8, space="PSUM") as pps:
                for ci in range(nchunks):
                    px0 = ci * CHUNK
                    cnt = min(CHUNK, PX - px0)
                    nblk = (cnt + 127) // 128
                    ins = []
                    for h in range(2):
                        it = pin.tile([128, CHUNK], f32, tag=f"in{h}")
                        nc.sync.dma_start(
                            it[:, :cnt],
                            bass.AP(img.tensor, h * 128 * PX + px0,
                                    [[PX, 128], [1, cnt]]))
                        ins.append(it)
                    stage = pst.tile([128, CHUNK * 2], f32, tag="st")
                    for b in range(nblk):
                        bc = min(128, cnt - b * 128)
                        pt = pps.tile([128, 256], f32, tag="ps")
                        for h in range(2):
                            nc.tensor.transpose(
                                out=pt[:bc, h * 128:(h + 1) * 128],
                                in_=ins[h][:, b * 128:b * 128 + bc],
                                identity=ident[:])
                        nc.vector.tensor_copy(
                            out=stage[:bc, b * 256:(b + 1) * 256], in_=pt[:bc, :])
                    # write r=0 part: pixels px < PX - W  (dst off = px*512)
                    # write r=1 part: pixels px >= W      (dst off = (px-W)*512 + 256)
                    for r in range(2):
                        lo = max(px0, W) if r == 1 else px0
                        hi = min(px0 + cnt, PX - W) if r == 0 else px0 + cnt
                        if hi <= lo:
                            continue
                        b0, b1 = (lo - px0) // 128, (hi - 1 - px0) // 128
                        for bseg0 in range(b0, b1 + 1):
                            # contiguous full-block run [bseg0..bseg_end] with equal
                            # partition-extent; emit per-block partial edges separately
                            pass
                        # simpler: emit one DMA per 128-block (<=8 per chunk)
                        for b in range(b0, b1 + 1):
                            s = max(lo, px0 + b * 128)
                            e = min(hi, px0 + b * 128 + min(128, cnt - b * 128))
                            if e <= s:
                                continue
                            p_off = s - (px0 + b * 128)   # partition start in block
                            n_p = e - s
                            dst_off = (s - r * W) * 512 + r * 256
                            nc.sync.dma_start(
                                bass.AP(qflat.tensor, dst_off, [[512, n_p], [1, 256]]),
                                stage[p_off:p_off + n_p, b * 256:(b + 1) * 256])

            # zero the pad rows of Q (px >= PX - W at r-slot granularity is
            # written, but flat rows [PX-W .. PX) of the padded tensor are not)
            with tc.tile_pool(name="pz", bufs=1) as pz:
                zt = pz.tile([128, 800], f32)
                nc.vector.memset(zt[:], 0.0)
                nc.sync.dma_start(
                    bass.AP(qflat.tensor, (PX - W) * 512, [[800, 128], [1, 800]]),
                    zt[:])

            # ---------------- Phase B: gather + bilinear ----------------
            qrows = bass.AP(qflat.tensor, 0, [[512, PX], [1, 512]])
            with tc.tile_pool(name="pb_io", bufs=1) as pio, \
                 tc.tile_pool(name="pb_g", bufs=6) as pg, \
                 tc.tile_pool(name="pb_t", bufs=6) as ptm, \
                 tc.tile_pool(name="pb_ob", bufs=1) as pob, \
                 tc.tile_pool(name="pb_ps", bufs=4, space="PSUM") as pps:
                idxt = pio.tile([128, R], i32)
                nc.sync.dma_start(idxt[:], idxg[:])
                wt = pio.tile([128, 4 * R], f32)
                nc.sync.dma_start(wt[:], wts[:])
                obuf = []
                for h in range(2):
                    ob = pob.tile([128, R * 128], f32, tag=f"ob{h}", name=f"ob{h}")
                    obuf.append(ob)
                for r in range(R):
                    g = pg.tile([128, 1024], f32, tag="g")
                    nc.gpsimd.indirect_dma_start(
                        out=g[:], out_offset=None, in_=qrows,
                        in_offset=bass.IndirectOffsetOnAxis(ap=idxt[:, r:r + 1], axis=0))
                    val = ptm.tile([128, 256], f32, tag="val")
                    acc = ptm.tile([128, 256], f32, tag="acc")
                    # weights order per point: [wy0*wx0, wy1*wx0, wy0*wx1, wy1*wx1]
                    nc.vector.tensor_scalar_mul(val[:], g[:, 0:256], wt[:, 4 * r:4 * r + 1])
                    nc.vector.tensor_scalar_mul(acc[:], g[:, 256:512], wt[:, 4 * r + 1:4 * r + 2])
                    nc.vector.tensor_add(val[:], val[:], acc[:])
                    nc.vector.tensor_scalar_mul(acc[:], g[:, 512:768], wt[:, 4 * r + 2:4 * r + 3])
                    nc.vector.tensor_add(val[:], val[:], acc[:])
                    nc.vector.tensor_scalar_mul(acc[:], g[:, 768:1024], wt[:, 4 * r + 3:4 * r + 4])
                    nc.vector.tensor_add(val[:], val[:], acc[:])
                    for h in range(2):
                        pt = pps.tile([128, 128], f32, tag=f"pt{h}")
                        nc.tensor.transpose(out=pt[:], in_=val[:, h * 128:(h + 1) * 128],
                                            identity=ident[:])
                        nc.vector.tensor_copy(out=obuf[h][:, r * 128:(r + 1) * 128],
                                              in_=pt[:])
                # output: obuf[h][c, m*196+pt] -> out[m, h*128+c, pt]
                for h in range(2):
                    nc.sync.dma_start(
                        bass.AP(out.tensor, h * 128 * NPT,
                                [[NPT, 128], [C * NPT, M], [1, NPT]]),
                        obuf[h][:, :M * NPT].rearrange("p (m t) -> p m t", m=M))
    nc.compile()
    return nc


def _runner(nc):
    install_neuronx_cc_hook()
    partition_name = nc.partition_id_tensor.name if nc.partition_id_tensor else None
    in_names, out_names, out_avals, zero_shapes = [], [], [], []
    for alloc in nc.m.functions[0].allocations:
        if not isinstance(alloc, mybir.MemoryLocationSet):
            continue
        name = alloc.memorylocations[0].name
        if alloc.kind == "ExternalInput":
            if name != partition_name:
                in_names.append(name)
        elif alloc.kind == "ExternalOutput":
            out_names.append(name)
            shape = tuple(alloc.tensor_shape)
            dtype = mybir.dt.np(alloc.dtype)
            out_avals.append(jax.core.ShapedArray(shape, dtype))
            zero_shapes.append((shape, dtype))
    n_params = len(in_names)
    all_in = in_names + out_names + ([partition_name] if partition_name else [])

    def _body(*args):
        operands = list(args)
        if partition_name is not None:
            operands.append(partition_id_tensor())
        return tuple(_bass_exec_p.bind(
            *operands, out_avals=tuple(out_avals), in_names=tuple(all_in),
            out_names=tuple(out_names), lowering_input_output_aliases=(),
            sim_require_finite=True, sim_require_nnan=True, nc=nc))

    devices = jax.devices()[:N_CORES]
    mesh = Mesh(np.asarray(devices), ("core",))
    nio = n_params + len(out_names)
    sharded = jax.jit(
        shard_map(_body, mesh=mesh, in_specs=(PartitionSpec("core"),) * nio,
                  out_specs=(PartitionSpec("core"),) * len(out_names), check_rep=False),
        keep_unused=True)

    def run(in_maps):
        global LAST_EXEC_S
        concat = [np.concatenate([np.asarray(m[n]) for m in in_maps], axis=0)
                  for n in in_names]
        concat += [np.zeros((N_CORES * s[0], *s[1:]), d) for s, d in zero_shapes]
        staged = jax.device_put(concat)
        for a in staged:
            a.block_until_ready()
        t0 = time.perf_counter()
        outs = sharded(*staged)
        for o in outs:
            o.block_until_ready()
        LAST_EXEC_S = time.perf_counter() - t0
        return [
            {n: np.asarray(outs[i]).reshape(N_CORES, *out_avals[i].shape)[c]
             for i, n in enumerate(out_names)}
            for c in range(N_CORES)
        ]
    return run


def _params(boxes_m):
    """boxes_m: [M,4] -> (pixidx [M,196] int32, w4 [M,196,4] f32)"""
    y1, x1, y2, x2 = boxes_m[:, 0], boxes_m[:, 1], boxes_m[:, 2], boxes_m[:, 3]
    hs = (y2 - y1) * (H - 1) / (CH - 1)
    ws = (x2 - x1) * (W - 1) / (CW - 1)
    ar = np.arange(CH, dtype=np.float32)
    iny = y1[:, None] * (H - 1) + ar[None, :] * hs[:, None]      # [M,14]
    inx = x1[:, None] * (W - 1) + ar[None, :] * ws[:, None]
    vy = ((iny >= 0) & (iny <= H - 1)).astype(np.float32)
    vx = ((inx >= 0) & (inx <= W - 1)).astype(np.float32)
    ys = np.clip(np.floor(iny), 0, H - 2)
    xs = np.clip(np.floor(inx), 0, W - 2)
    wy1 = (iny - ys).astype(np.float32) * vy
    wy0 = (1.0 - (iny - ys)).astype(np.float32) * vy
    wx1 = (inx - xs).astype(np.float32) * vx
    wx0 = (1.0 - (inx - xs)).astype(np.float32) * vx
    # clip wy1/wx1 to [0,1]? when iny in [198,199]: ys=198, iny-ys in [0,1] ok;
    # iny=199 -> wy1=1 exact. invalid -> masked to 0.
    pix = (ys[:, :, None] * W + xs[:, None, :]).reshape(-1, NPT).astype(np.int32)
    w4 = np.empty((boxes_m.shape[0], NPT, 4), np.float32)
    w4[:, :, 0] = (wy0[:, :, None] * wx0[:, None, :]).reshape(-1, NPT)
    w4[:, :, 1] = (wy1[:, :, None] * wx0[:, None, :]).reshape(-1, NPT)
    w4[:, :, 2] = (wy0[:, :, None] * wx1[:, None, :]).reshape(-1, NPT)
    w4[:, :, 3] = (wy1[:, :, None] * wx1[:, None, :]).reshape(-1, NPT)
    return pix, w4


def kernel(image, boxes, box_indices):
    image = np.asarray(image, dtype=np.float32)
    boxes = np.asarray(boxes, dtype=np.float32)
    box_indices = np.asarray(box_indices, dtype=np.int32)
    N = boxes.shape[0]
    groups = [np.nonzero(box_indices == b)[0] for b in range(N_CORES)]
    M = max(1, max(len(g) for g in groups))
    R = (M * NPT + 127) // 128

    key = M
    if key not in _cache:
        nc = _build(M)
        _cache[key] = _runner(nc)
    run = _cache[key]

    in_maps = []
    for b in range(N_CORES):
        ids = groups[b]
        bx = np.zeros((M, 4), np.float32)
        bx[:len(ids)] = boxes[ids]
        pix, w4 = _params(bx)                       # [M,196], [M,196,4]
        npts = M * NPT
        pix_p = np.zeros(R * 128, np.int32)
        w4_p = np.zeros((R * 128, 4), np.float32)
        pix_p[:npts] = pix.reshape(-1)
        w4_p[:npts] = w4.reshape(-1, 4)
        # point g lives at (partition s=g%128, round r=g//128)
        idx_t = pix_p.reshape(R, 128).T.copy()       # [128, R]
        w_t = w4_p.reshape(R, 128, 4).transpose(1, 0, 2).reshape(128, 4 * R).copy()
        in_maps.append({
            "img": image[b].reshape(C, PX),
            "idxg": idx_t,
            "wts": w_t,
        })
    res = run(in_maps)
    out = np.empty((N, C, CH, CW), np.float32)
    for b in range(N_CORES):
        ids = groups[b]
        if len(ids):
            out[ids] = res[b]["out"][:len(ids)].reshape(len(ids), C, CH, CW)
    return out



# revision 4
# speedup vs baseline: 134.0839x; 134.0839x over previous
"""CropAndResize (TF-style, bilinear, extrap=0) on 8 trn2 NeuronCores.

Sharding: data-parallel over batch B=8 (core b owns image[b]); boxes grouped by
their batch index (sharding_hint option 2). Each core:
  Phase A: CHW -> Q layout in DRAM, Q[y, x, r, c] = img[y+r, x, c]
           (paired rows, channels innermost) via PE transposes. 199x200x2x256 f32.
  Phase B: per 128 sample points, one indirect-DMA gather: descriptor s fetches
           the 4KB block Q[ys, xs, :, :] (the 4 bilinear corner pixels x 256ch),
           DVE applies the 4 bilinear weights (per-partition scalars),
           PE transposes [pt, c] -> [c, pt], result accumulates in SBUF,
           two big DMAs write [c, pts] -> out[m, c, 14, 14].
Host only: grouping boxes by box_indices, bilinear index/weight precompute
(O(N*14) floats), and unshard of outputs.
"""
import sys, os, time
sys.path.insert(0, "/opt/trn_rl_repo")
import numpy as np

import concourse.bass as bass
import concourse.bacc as bacc
import concourse.tile as tile
import concourse.mybir as mybir
from concourse.masks import make_identity
import jax
from jax.sharding import Mesh, PartitionSpec
from jax.experimental.shard_map import shard_map
from concourse.bass2jax import _bass_exec_p, install_neuronx_cc_hook, partition_id_tensor

N_CORES = 8
C, H, W = 256, 200, 200
CH, CW = 14, 14
NPT = CH * CW                     # 196 points per box
PX = H * W                        # 40000 pixels
QROWS = PX - W                    # valid block start pixels: ys<=198 -> idx <= 39798

_cache = {}
LAST_EXEC_S = None
LAST_NC = None
LAST_IN_MAPS = None


def _build(M):
    """Build + compile the SPMD program for M boxes per core. Returns runner."""
    R = (M * NPT + 127) // 128    # gather rounds (128 points each)
    nc = bacc.Bacc("TRN2", target_bir_lowering=False, debug=False, num_devices=N_CORES)
    f32, i32 = mybir.dt.float32, mybir.dt.int32

    img = nc.dram_tensor("img", [C, PX], f32, kind="ExternalInput").ap()
    idxg = nc.dram_tensor("idxg", [128, R], i32, kind="ExternalInput").ap()
    wts = nc.dram_tensor("wts", [128, 4 * R], f32, kind="ExternalInput").ap()
    out = nc.dram_tensor("out", [M, C, NPT], f32, kind="ExternalOutput").ap()
    # Q scratch: flat (H-1)*W*2*C elems; viewed as rows of 512 f32 for the gather
    qflat = nc.dram_tensor("qscratch", [PX * 2 * C], f32, kind="Internal").ap()

    CHUNK = 1024
    nchunks = (PX + CHUNK - 1) // CHUNK

    with tile.TileContext(nc) as tc:
        with tc.tile_pool(name="ident", bufs=1) as ipool:
            ident = ipool.tile([128, 128], f32)
            make_identity(nc, ident[:])

            # ---------------- Phase A: build Q ----------------
            with tc.tile_pool(name="pa_in", bufs=3) as pin, \
                 tc.tile_pool(name="pa_st", bufs=3) as pst, \
                 tc.tile_pool(name="pa_ps", bufs=8, space="PSUM") as pps:
                for ci in range(nchunks):
                    px0 = ci * CHUNK
                    cnt = min(CHUNK, PX - px0)
                    nblk = (cnt + 127) // 128
                    ins = []
                    for h in range(2):
                        it = pin.tile([128, CHUNK], f32, tag=f"in{h}")
                        nc.sync.dma_start(
                            it[:, :cnt],
                            bass.AP(img.tensor, h * 128 * PX + px0,
                                    [[PX, 128], [1, cnt]]))
                        ins.append(it)
                    stage = pst.tile([128, CHUNK * 2], f32, tag="st")
                    for b in range(nblk):
                        bc = min(128, cnt - b * 128)
                        pt = pps.tile([128, 256], f32, tag="ps")
                        for h in range(2):
                            nc.tensor.transpose(
                                out=pt[:bc, h * 128:(h + 1) * 128],
                                in_=ins[h][:, b * 128:b * 128 + bc],
                                identity=ident[:])
                        nc.vector.tensor_copy(
                            out=stage[:bc, b * 256:(b + 1) * 256], in_=pt[:bc, :])
                    # write r=0 part: pixels px < PX - W  (dst off = px*512)
                    # write r=1 part: pixels px >= W      (dst off = (px-W)*512 + 256)
                    for r in range(2):
                        lo = max(px0, W) if r == 1 else px0
                        hi = min(px0 + cnt, PX - W) if r == 0 else px0 + cnt
                        if hi <= lo:
                            continue
                        b0, b1 = (lo - px0) // 128, (hi - 1 - px0) // 128
                        for bseg0 in range(b0, b1 + 1):
                            # contiguous full-block run [bseg0..bseg_end] with equal
                            # partition-extent; emit per-block partial edges separately
                            pass
                        # simpler: emit one DMA per 128-block (<=8 per chunk)
                        for b in range(b0, b1 + 1):
                            s = max(lo, px0 + b * 128)
                            e = min(hi, px0 + b * 128 + min(128, cnt - b * 128))
                            if e <= s:
                                continue
                            p_off = s - (px0 + b * 128)   # partition start in block
                            n_p = e - s
                            dst_off = (s - r * W) * 512 + r * 256
                            nc.sync.dma_start(
                                bass.AP(qflat.tensor, dst_off, [[512, n_p], [1, 256]]),
                                stage[p_off:p_off + n_p, b * 256:(b + 1) * 256])

            # zero the pad rows of Q (px >= PX - W at r-slot granularity is
            # written, but flat rows [PX-W .. PX) of the padded tensor are not)
            with tc.tile_pool(name="pz", bufs=1) as pz:
                zt = pz.tile([128, 800], f32)
                nc.vector.memset(zt[:], 0.0)
                nc.sync.dma_start(
                    bass.AP(qflat.tensor, (PX - W) * 512, [[800, 128], [1, 800]]),
                    zt[:])

            # ---------------- Phase B: gather + bilinear ----------------
            qrows = bass.AP(qflat.tensor, 0, [[512, PX], [1, 512]])
            with tc.tile_pool(name="pb_io", bufs=1) as pio, \
                 tc.tile_pool(name="pb_g", bufs=6) as pg, \
                 tc.tile_pool(name="pb_t", bufs=6) as ptm, \
                 tc.tile_pool(name="pb_ob", bufs=1) as pob, \
                 tc.tile_pool(name="pb_ps", bufs=4, space="PSUM") as pps:
                idxt = pio.tile([128, R], i32)
                nc.sync.dma_start(idxt[:], idxg[:])
                wt = pio.tile([128, 4 * R], f32)
                nc.sync.dma_start(wt[:], wts[:])
                obuf = []
                for h in range(2):
                    ob = pob.tile([128, R * 128], f32, tag=f"ob{h}", name=f"ob{h}")
                    obuf.append(ob)
                for r in range(R):
                    g = pg.tile([128, 1024], f32, tag="g")
                    nc.gpsimd.indirect_dma_start(
                        out=g[:], out_offset=None, in_=qrows,
                        in_offset=bass.IndirectOffsetOnAxis(ap=idxt[:, r:r + 1], axis=0))
                    val = ptm.tile([128, 256], f32, tag="val")
                    acc = ptm.tile([128, 256], f32, tag="acc")
                    # weights order per point: [wy0*wx0, wy1*wx0, wy0*wx1, wy1*wx1]
                    nc.vector.tensor_scalar_mul(val[:], g[:, 0:256], wt[:, 4 * r:4 * r + 1])
                    nc.vector.tensor_scalar_mul(acc[:], g[:, 256:512], wt[:, 4 * r + 1:4 * r + 2])
                    nc.vector.tensor_add(val[:], val[:], acc[:])
                    nc.vector.tensor_scalar_mul(acc[:], g[:, 512:768], wt[:, 4 * r + 2:4 * r + 3])
                    nc.vector.tensor_add(val[:], val[:], acc[:])
                    nc.vector.tensor_scalar_mul(acc[:], g[:, 768:1024], wt[:, 4 * r + 3:4 * r + 4])
                    nc.vector.tensor_add(val[:], val[:], acc[:])
                    for h in range(2):
                        pt = pps.tile([128, 128], f32, tag=f"pt{h}")
                        nc.tensor.transpose(out=pt[:], in_=val[:, h * 128:(h + 1) * 128],
                                            identity=ident[:])
                        nc.vector.tensor_copy(out=obuf[h][:, r * 128:(r + 1) * 128],
                                              in_=pt[:])
                # output: obuf[h][c, m*196+pt] -> out[m, h*128+c, pt]
                for h in range(2):
                    nc.sync.dma_start(
                        bass.AP(out.tensor, h * 128 * NPT,
                                [[NPT, 128], [C * NPT, M], [1, NPT]]),
                        obuf[h][:, :M * NPT].rearrange("p (m t) -> p m t", m=M))
    nc.compile()
    return nc


def _runner(nc):
    install_neuronx_cc_hook()
    partition_name = nc.partition_id_tensor.name if nc.partition_id_tensor else None
    in_names, out_names, out_avals, zero_shapes = [], [], [], []
    for alloc in nc.m.functions[0].allocations:
        if not isinstance(alloc, mybir.MemoryLocationSet):
            continue
        name = alloc.memorylocations[0].name
        if alloc.kind == "ExternalInput":
            if name != partition_name:
                in_names.append(name)
        elif alloc.kind == "ExternalOutput":
            out_names.append(name)
            shape = tuple(alloc.tensor_shape)
            dtype = mybir.dt.np(alloc.dtype)
            out_avals.append(jax.core.ShapedArray(shape, dtype))
            zero_shapes.append((shape, dtype))
    n_params = len(in_names)
    all_in = in_names + out_names + ([partition_name] if partition_name else [])

    def _body(*args):
        operands = list(args)
        if partition_name is not None:
            operands.append(partition_id_tensor())
        return tuple(_bass_exec_p.bind(
            *operands, out_avals=tuple(out_avals), in_names=tuple(all_in),
            out_names=tuple(out_names), lowering_input_output_aliases=(),
            sim_require_finite=True, sim_require_nnan=True, nc=nc))

    devices = jax.devices()[:N_CORES]
    mesh = Mesh(np.asarray(devices), ("core",))
    nio = n_params + len(out_names)
    sharded = jax.jit(
        shard_map(_body, mesh=mesh, in_specs=(PartitionSpec("core"),) * nio,
                  out_specs=(PartitionSpec("core"),) * len(out_names), check_rep=False),
        keep_unused=True)

    def run(in_maps):
        global LAST_EXEC_S
        concat = [np.concatenate([np.asarray(m[n]) for m in in_maps], axis=0)
                  for n in in_names]
        concat += [np.zeros((N_CORES * s[0], *s[1:]), d) for s, d in zero_shapes]
        staged = jax.device_put(concat)
        for a in staged:
            a.block_until_ready()
        t0 = time.perf_counter()
        outs = sharded(*staged)
        for o in outs:
            o.block_until_ready()
        LAST_EXEC_S = time.perf_counter() - t0
        return [
            {n: np.asarray(outs[i]).reshape(N_CORES, *out_avals[i].shape)[c]
             for i, n in enumerate(out_names)}
            for c in range(N_CORES)
        ]
    return run


def _params(boxes_m):
    """boxes_m: [M,4] -> (pixidx [M,196] int32, w4 [M,196,4] f32)"""
    y1, x1, y2, x2 = boxes_m[:, 0], boxes_m[:, 1], boxes_m[:, 2], boxes_m[:, 3]
    hs = (y2 - y1) * (H - 1) / (CH - 1)
    ws = (x2 - x1) * (W - 1) / (CW - 1)
    ar = np.arange(CH, dtype=np.float32)
    iny = y1[:, None] * (H - 1) + ar[None, :] * hs[:, None]      # [M,14]
    inx = x1[:, None] * (W - 1) + ar[None, :] * ws[:, None]
    vy = ((iny >= 0) & (iny <= H - 1)).astype(np.float32)
    vx = ((inx >= 0) & (inx <= W - 1)).astype(np.float32)
    ys = np.clip(np.floor(iny), 0, H - 2)
    xs = np.clip(np.floor(inx), 0, W - 2)
    wy1 = (iny - ys).astype(np.float32) * vy
    wy0 = (1.0 - (iny - ys)).astype(np.float32) * vy
    wx1 = (inx - xs).astype(np.float32) * vx
    wx0 = (1.0 - (inx - xs)).astype(np.float32) * vx
    # clip wy1/wx1 to [0,1]? when iny in [198,199]: ys=198, iny-ys in [0,1] ok;
    # iny=199 -> wy1=1 exact. invalid -> masked to 0.
    pix = (ys[:, :, None] * W + xs[:, None, :]).reshape(-1, NPT).astype(np.int32)
    w4 = np.empty((boxes_m.shape[0], NPT, 4), np.float32)
    w4[:, :, 0] = (wy0[:, :, None] * wx0[:, None, :]).reshape(-1, NPT)
    w4[:, :, 1] = (wy1[:, :, None] * wx0[:, None, :]).reshape(-1, NPT)
    w4[:, :, 2] = (wy0[:, :, None] * wx1[:, None, :]).reshape(-1, NPT)
    w4[:, :, 3] = (wy1[:, :, None] * wx1[:, None, :]).reshape(-1, NPT)
    return pix, w4


def kernel(image, boxes, box_indices):
    image = np.asarray(image, dtype=np.float32)
    boxes = np.asarray(boxes, dtype=np.float32)
    box_indices = np.asarray(box_indices, dtype=np.int32)
    N = boxes.shape[0]
    groups = [np.nonzero(box_indices == b)[0] for b in range(N_CORES)]
    M = max(1, max(len(g) for g in groups))
    R = (M * NPT + 127) // 128

    global LAST_NC, LAST_IN_MAPS
    key = M
    if key not in _cache:
        nc = _build(M)
        _cache[key] = (_runner(nc), nc)
    run, LAST_NC = _cache[key]

    in_maps = []
    for b in range(N_CORES):
        ids = groups[b]
        bx = np.zeros((M, 4), np.float32)
        bx[:len(ids)] = boxes[ids]
        pix, w4 = _params(bx)                       # [M,196], [M,196,4]
        npts = M * NPT
        pix_p = np.zeros(R * 128, np.int32)
        w4_p = np.zeros((R * 128, 4), np.float32)
        pix_p[:npts] = pix.reshape(-1)
        w4_p[:npts] = w4.reshape(-1, 4)
        # point g lives at (partition s=g%128, round r=g//128)
        idx_t = pix_p.reshape(R, 128).T.copy()       # [128, R]
        w_t = w4_p.reshape(R, 128, 4).transpose(1, 0, 2).reshape(128, 4 * R).copy()
        in_maps.append({
            "img": image[b].reshape(C, PX),
            "idxg": idx_t,
            "wts": w_t,
        })
    LAST_IN_MAPS = in_maps
    res = run(in_maps)
    out = np.empty((N, C, CH, CW), np.float32)
    for b in range(N_CORES):
        ids = groups[b]
        if len(ids):
            out[ids] = res[b]["out"][:len(ids)].reshape(len(ids), C, CH, CW)
    return out



# revision 5
# speedup vs baseline: 190.8484x; 1.4234x over previous
"""CropAndResize on 8 trn2 NeuronCores. v3: baseline f32 Q/gather (HW-proven
4KB descriptors) + coalesced Phase-A writes + PE diag-matmul bilinear reduce
(bf16, after on-chip cast) + bf16 contiguous output.
"""
import sys, os, time
sys.path.insert(0, "/opt/trn_rl_repo")
import numpy as np

import concourse.bass as bass
import concourse.bacc as bacc
import concourse.tile as tile
import concourse.mybir as mybir
from concourse.masks import make_identity
import jax
from jax.sharding import Mesh, PartitionSpec
from jax.experimental.shard_map import shard_map
from concourse.bass2jax import _bass_exec_p, install_neuronx_cc_hook, partition_id_tensor

N_CORES = 8
C, H, W = 256, 200, 200
CH, CW = 14, 14
NPT = CH * CW
PX = H * W

_cache = {}
LAST_EXEC_S = None
LAST_NC = None
LAST_IN_MAPS = None


def _rounds(M):
    return (M * NPT + 127) // 128


def _build(M):
    R = _rounds(M)
    nc = bacc.Bacc("TRN2", target_bir_lowering=False, debug=False, num_devices=N_CORES)
    f32, i32 = mybir.dt.float32, mybir.dt.int32
    bf16 = mybir.dt.bfloat16

    img = nc.dram_tensor("img", [C, PX], f32, kind="ExternalInput").ap()
    idxg = nc.dram_tensor("idxg", [128, R], i32, kind="ExternalInput").ap()
    wts = nc.dram_tensor("wts", [128, 4 * R], f32, kind="ExternalInput").ap()
    # [h, c, m, pt]: each SBUF partition (a channel) writes one contiguous run
    out = nc.dram_tensor("out", [2, 128, M, NPT], bf16, kind="ExternalOutput").ap()
    # dup-Q f32: row p (1KB... 512 f32 = 2KB) = [img[y,x,:], img[y+1,x,:]]
    qflat = nc.dram_tensor("qscratch", [PX * 2 * C], f32, kind="Internal").ap()

    CHUNK = 2048
    nchunks = (PX + CHUNK - 1) // CHUNK

    with tile.TileContext(nc) as tc:
        with tc.tile_pool(name="ident", bufs=1) as ipool:
            ident32 = ipool.tile([128, 128], f32, tag="i32")
            make_identity(nc, ident32[:])
            ident = ipool.tile([128, 128], bf16, tag="i16")
            nc.vector.tensor_copy(out=ident[:], in_=ident32[:])

            # ---------------- Phase A: build dup-Q f32 ----------------
            with tc.tile_pool(name="pa_in", bufs=3) as pin, \
                 tc.tile_pool(name="pa_st", bufs=3) as pst, \
                 tc.tile_pool(name="pa_ps", bufs=8, space="PSUM") as pps:
                for ci in range(nchunks):
                    px0 = ci * CHUNK
                    cnt = min(CHUNK, PX - px0)
                    nblk = (cnt + 127) // 128
                    ins = []
                    for h in range(2):
                        it = pin.tile([128, CHUNK], f32, tag=f"in{h}")
                        nc.sync.dma_start(
                            it[:, :cnt],
                            bass.AP(img.tensor, h * 128 * PX + px0,
                                    [[PX, 128], [1, cnt]]))
                        ins.append(it)
                    stage = pst.tile([128, CHUNK * 2], f32, tag="st")
                    for b in range(nblk):
                        bc = min(128, cnt - b * 128)
                        pt = pps.tile([128, 256], f32, tag="ps")
                        for h in range(2):
                            nc.tensor.transpose(
                                out=pt[:bc, h * 128:(h + 1) * 128],
                                in_=ins[h][:, b * 128:b * 128 + bc],
                                identity=ident32[:])
                        dst = stage[:bc, b * 256:(b + 1) * 256]
                        if b % 2 == 0:
                            nc.vector.tensor_copy(out=dst, in_=pt[:bc, :])
                        else:
                            nc.scalar.activation(
                                out=dst, in_=pt[:bc, :],
                                func=mybir.ActivationFunctionType.Copy)
                    # r=0: px < PX-W at px*512; r=1: px >= W at (px-W)*512+256
                    # coalesce full-block runs into one 3D-AP DMA per (chunk, r)
                    for r in range(2):
                        lo = max(px0, W) if r == 1 else px0
                        hi = min(px0 + cnt, PX - W) if r == 0 else px0 + cnt
                        if hi <= lo:
                            continue
                        bf0 = (lo - px0 + 127) // 128
                        bf1 = min((hi - px0) // 128, cnt // 128)
                        if bf1 > bf0:
                            base = (px0 + 128 * bf0 - r * W) * 512 + r * 256
                            nc.sync.dma_start(
                                bass.AP(qflat.tensor, base,
                                        [[512, 128], [128 * 512, bf1 - bf0],
                                         [1, 256]]),
                                stage[:, bf0 * 256:bf1 * 256].rearrange(
                                    "p (n c) -> p n c", c=256))
                        b0, b1 = (lo - px0) // 128, (hi - 1 - px0) // 128
                        for b in range(b0, b1 + 1):
                            if bf0 <= b < bf1:
                                continue
                            s = max(lo, px0 + b * 128)
                            e = min(hi, px0 + b * 128 + min(128, cnt - b * 128))
                            if e <= s:
                                continue
                            p_off = s - (px0 + b * 128)
                            n_p = e - s
                            dst_off = (s - r * W) * 512 + r * 256
                            nc.sync.dma_start(
                                bass.AP(qflat.tensor, dst_off,
                                        [[512, n_p], [1, 256]]),
                                stage[p_off:p_off + n_p, b * 256:(b + 1) * 256])

            with tc.tile_pool(name="pz", bufs=1) as pz:
                zt = pz.tile([128, 800], f32)
                nc.vector.memset(zt[:], 0.0)
                nc.sync.dma_start(
                    bass.AP(qflat.tensor, (PX - W) * 512, [[800, 128], [1, 800]]),
                    zt[:])

            # ------------- Phase B: f32 gather, cast, PE diag reduce -------------
            qrows = bass.AP(qflat.tensor, 0, [[512, PX], [1, 512]])
            with tc.tile_pool(name="pb_io", bufs=1) as pio, \
                 tc.tile_pool(name="pb_g", bufs=4) as pg, \
                 tc.tile_pool(name="pb_d", bufs=8) as pd, \
                 tc.tile_pool(name="pb_ob", bufs=1) as pob, \
                 tc.tile_pool(name="pb_ps", bufs=4, space="PSUM") as pps:
                idxt = pio.tile([128, R], i32)
                nc.sync.dma_start(idxt[:], idxg[:])
                wt = pio.tile([128, 4 * R], f32)
                nc.sync.dma_start(wt[:], wts[:])
                obuf = []
                for h in range(2):
                    ob = pob.tile([128, R * 128], bf16, tag=f"ob{h}", name=f"ob{h}")
                    obuf.append(ob)
                for r in range(R):
                    g = pg.tile([128, 1024], f32, tag="g")
                    nc.gpsimd.indirect_dma_start(
                        out=g[:], out_offset=None, in_=qrows,
                        in_offset=bass.IndirectOffsetOnAxis(
                            ap=idxt[:, r:r + 1], axis=0))
                    gb = pg.tile([128, 1024], bf16, tag="gb")
                    nc.vector.tensor_copy(out=gb[:, :512], in_=g[:, :512])
                    nc.scalar.activation(
                        out=gb[:, 512:], in_=g[:, 512:],
                        func=mybir.ActivationFunctionType.Copy)
                    dks = []
                    for k in range(4):
                        dk = pd.tile([128, 128], bf16, tag=f"d{k}")
                        if k % 2 == 0:
                            nc.vector.tensor_scalar_mul(
                                dk[:], ident[:], wt[:, 4 * r + k:4 * r + k + 1])
                        else:
                            nc.scalar.activation(
                                out=dk[:], in_=ident[:],
                                func=mybir.ActivationFunctionType.Copy,
                                scale=wt[:, 4 * r + k:4 * r + k + 1])
                        dks.append(dk)
                    for h in range(2):
                        pt = pps.tile([128, 128], f32, tag=f"pt{h}")
                        for k in range(4):
                            nc.tensor.matmul(
                                pt[:],
                                gb[:, k * 256 + h * 128:k * 256 + (h + 1) * 128],
                                dks[k][:], start=(k == 0), stop=(k == 3))
                        dst = obuf[h][:, r * 128:(r + 1) * 128]
                        if h == 0:
                            nc.vector.tensor_copy(out=dst, in_=pt[:])
                        else:
                            nc.scalar.activation(
                                out=dst, in_=pt[:],
                                func=mybir.ActivationFunctionType.Copy)
                for h in range(2):
                    nc.sync.dma_start(
                        bass.AP(out.tensor, h * 128 * M * NPT,
                                [[M * NPT, 128], [1, M * NPT]]),
                        obuf[h][:, :M * NPT])
    nc.compile()
    return nc


def _runner(nc):
    install_neuronx_cc_hook()
    partition_name = nc.partition_id_tensor.name if nc.partition_id_tensor else None
    in_names, out_names, out_avals, zero_shapes = [], [], [], []
    for alloc in nc.m.functions[0].allocations:
        if not isinstance(alloc, mybir.MemoryLocationSet):
            continue
        name = alloc.memorylocations[0].name
        if alloc.kind == "ExternalInput":
            if name != partition_name:
                in_names.append(name)
        elif alloc.kind == "ExternalOutput":
            out_names.append(name)
            shape = tuple(alloc.tensor_shape)
            dtype = mybir.dt.np(alloc.dtype)
            out_avals.append(jax.core.ShapedArray(shape, dtype))
            zero_shapes.append((shape, dtype))
    n_params = len(in_names)
    all_in = in_names + out_names + ([partition_name] if partition_name else [])

    def _body(*args):
        operands = list(args)
        if partition_name is not None:
            operands.append(partition_id_tensor())
        return tuple(_bass_exec_p.bind(
            *operands, out_avals=tuple(out_avals), in_names=tuple(all_in),
            out_names=tuple(out_names), lowering_input_output_aliases=(),
            sim_require_finite=True, sim_require_nnan=True, nc=nc))

    devices = jax.devices()[:N_CORES]
    mesh = Mesh(np.asarray(devices), ("core",))
    nio = n_params + len(out_names)
    sharded = jax.jit(
        shard_map(_body, mesh=mesh, in_specs=(PartitionSpec("core"),) * nio,
                  out_specs=(PartitionSpec("core"),) * len(out_names),
                  check_rep=False),
        keep_unused=True)

    def run(in_maps):
        global LAST_EXEC_S
        concat = [np.concatenate([np.asarray(m[n]) for m in in_maps], axis=0)
                  for n in in_names]
        concat += [np.zeros((N_CORES * s[0], *s[1:]), d) for s, d in zero_shapes]
        staged = jax.device_put(concat)
        for a in staged:
            a.block_until_ready()
        t0 = time.perf_counter()
        outs = sharded(*staged)
        for o in outs:
            o.block_until_ready()
        LAST_EXEC_S = time.perf_counter() - t0
        return [
            {n: np.asarray(outs[i]).reshape(N_CORES, *out_avals[i].shape)[c]
             for i, n in enumerate(out_names)}
            for c in range(N_CORES)
        ]
    return run


def _params(boxes_m):
    y1, x1, y2, x2 = boxes_m[:, 0], boxes_m[:, 1], boxes_m[:, 2], boxes_m[:, 3]
    hs = (y2 - y1) * (H - 1) / (CH - 1)
    ws = (x2 - x1) * (W - 1) / (CW - 1)
    ar = np.arange(CH, dtype=np.float32)
    iny = y1[:, None] * (H - 1) + ar[None, :] * hs[:, None]
    inx = x1[:, None] * (W - 1) + ar[None, :] * ws[:, None]
    vy = ((iny >= 0) & (iny <= H - 1)).astype(np.float32)
    vx = ((inx >= 0) & (inx <= W - 1)).astype(np.float32)
    ys = np.clip(np.floor(iny), 0, H - 2)
    xs = np.clip(np.floor(inx), 0, W - 2)
    wy1 = (iny - ys).astype(np.float32) * vy
    wy0 = (1.0 - (iny - ys)).astype(np.float32) * vy
    wx1 = (inx - xs).astype(np.float32) * vx
    wx0 = (1.0 - (inx - xs)).astype(np.float32) * vx
    pix = (ys[:, :, None] * W + xs[:, None, :]).reshape(-1, NPT).astype(np.int32)
    w4 = np.empty((boxes_m.shape[0], NPT, 4), np.float32)
    w4[:, :, 0] = (wy0[:, :, None] * wx0[:, None, :]).reshape(-1, NPT)
    w4[:, :, 1] = (wy1[:, :, None] * wx0[:, None, :]).reshape(-1, NPT)
    w4[:, :, 2] = (wy0[:, :, None] * wx1[:, None, :]).reshape(-1, NPT)
    w4[:, :, 3] = (wy1[:, :, None] * wx1[:, None, :]).reshape(-1, NPT)
    return pix, w4


def kernel(image, boxes, box_indices):
    image = np.asarray(image, dtype=np.float32)
    boxes = np.asarray(boxes, dtype=np.float32)
    box_indices = np.asarray(box_indices, dtype=np.int32)
    N = boxes.shape[0]
    groups = [np.nonzero(box_indices == b)[0] for b in range(N_CORES)]
    M = max(1, max(len(g) for g in groups))
    R = _rounds(M)

    global LAST_NC, LAST_IN_MAPS
    key = M
    if key not in _cache:
        nc = _build(M)
        _cache[key] = (_runner(nc), nc)
    run, LAST_NC = _cache[key]

    in_maps = []
    for b in range(N_CORES):
        ids = groups[b]
        bx = np.zeros((M, 4), np.float32)
        bx[:len(ids)] = boxes[ids]
        pix, w4 = _params(bx)
        npts = M * NPT
        pix_p = np.zeros(R * 128, np.int32)
        w4_p = np.zeros((R * 128, 4), np.float32)
        pix_p[:npts] = pix.reshape(-1)
        w4_p[:npts] = w4.reshape(-1, 4)
        idx_t = pix_p.reshape(R, 128).T.copy()
        w_t = w4_p.reshape(R, 128, 4).transpose(1, 0, 2).reshape(128, 4 * R).copy()
        in_maps.append({
            "img": image[b].reshape(C, PX),
            "idxg": idx_t,
            "wts": w_t,
        })
    LAST_IN_MAPS = in_maps
    res = run(in_maps)
    out = np.empty((N, C, CH, CW), np.float32)
    for b in range(N_CORES):
        ids = groups[b]
        if len(ids):
            ob = res[b]["out"].reshape(2, 128, M, NPT).transpose(2, 0, 1, 3)
            out[ids] = ob[:len(ids)].astype(np.float32).reshape(
                len(ids), C, CH, CW)
    return out
